# revision 1
# baseline (speedup 1.0000x reference)
"""DCN block kernel for Trainium2 (8 NeuronCores, data-parallel over batch).

Math (per batch b, plane c):
  z   = conv3x3(x, w_off) + b_off                  (64 offset logits)
  d   = sigmoid(z) - 0.5   in (-.5, .5)            (pixel displacement)
  sample at (r - dy, c - dx) bilinear w/ reflect   (|d| < .5 => 3x3 support!)
  y   = conv3x3(sampled, w_dcn) + b_dcn

Because |d| < 0.5 the bilinear gather only touches the 3x3 neighborhood, so it
is computed gather-free as
  H(sigma)  = x + dxt*AR + |dxt|*BR     (AR = x(c-1)-x(c+1), BR = x(c-1)+x(c+1)-2x)
  out = H0 + dyt*(Hm-Hp) + |dyt|*(Hm+Hp-2H0)
with dxt = d/2. With reflect-consistent fixups at image border rows/cols this
is exact.

Layout: 4 image row-quarters stacked on partition groups [4 x 32ch]; convs run
as 4 concurrent row-tiled matmul streams (tile_position), K=32, 9 taps
accumulating in PSUM; elementwise sampling runs on [128, fd] bf16 tiles.
"""

import math
from contextlib import ExitStack

import ml_dtypes
import numpy as np

import concourse.bacc as bacc
import concourse.bass as bass
import concourse.mybir as mybir
import concourse.tile as tile

BF16 = mybir.dt.bfloat16
F32 = mybir.dt.float32
AF = mybir.ActivationFunctionType
OP = mybir.AluOpType

N_CORES = 8
C = 32          # input/output channels per plane set
OC2 = 64        # offset logits (2 per plane)


class Cfg:
    def __init__(self, H=384, nr=8):
        self.H = H
        self.W = H
        self.WP = self.W + 2          # padded row: [pad, 0..W-1, pad]
        self.QH = H // 4              # rows per quarter
        assert self.QH % nr == 0
        self.nr = nr                  # output rows per quarter per slab
        self.nslab = self.QH // nr


def _f(ap):
    """Flatten free dims of a 3d tile AP to [P, fd]."""
    return ap.rearrange("p a b -> p (a b)")


def build_nc(cfg: Cfg, finalize=True):
    nc = bacc.Bacc()
    H, W, WP, nr = cfg.H, cfg.W, cfg.WP, cfg.nr

    x_in = nc.declare_dram_parameter("x", [C, H + 4, W], BF16, isOutput=False)
    woff_in = nc.declare_dram_parameter("woff", [128, 9 * OC2], BF16, isOutput=False)
    wdcn_in = nc.declare_dram_parameter("wdcn", [128, 9 * C], BF16, isOutput=False)
    boff_in = nc.declare_dram_parameter("boff", [128, 1], F32, isOutput=False)
    bdcn_in = nc.declare_dram_parameter("bdcn", [128, 1], F32, isOutput=False)
    y_out = nc.declare_dram_parameter("y", [C, H, W], F32, isOutput=True)

    with tile.TileContext(nc) as tc, ExitStack() as ctx:
        fold_sem = ctx.enter_context(nc.semaphore("fold_sem"))
        fold_cnt = [0]
        store_sem = ctx.enter_context(nc.semaphore("store_sem"))
        store_cnt = [0]
        consts = ctx.enter_context(tc.tile_pool(name="consts", bufs=1))
        xpool = ctx.enter_context(tc.tile_pool(name="xp", bufs=1))
        abpool = ctx.enter_context(tc.tile_pool(name="abp", bufs=1))
        spool = ctx.enter_context(tc.tile_pool(name="sp", bufs=1))
        mpool = ctx.enter_context(tc.tile_pool(name="mp", bufs=1))
        hpool = ctx.enter_context(tc.tile_pool(name="hp", bufs=1))
        ospool = ctx.enter_context(tc.tile_pool(name="osp", bufs=1))
        ocpool = ctx.enter_context(tc.tile_pool(name="ocp", bufs=2))
        zpool = ctx.enter_context(tc.tile_pool(name="zp", bufs=2, space="PSUM"))
        opool = ctx.enter_context(tc.tile_pool(name="op", bufs=2, space="PSUM"))

        WOFF = consts.tile([128, 9, OC2], BF16)
        nc.sync.dma_start(out=_f(WOFF), in_=woff_in[:])
        WDCN = consts.tile([128, 9, C], BF16)
        nc.sync.dma_start(out=_f(WDCN), in_=wdcn_in[:])
        BOFF = consts.tile([128, 1], F32)
        nc.sync.dma_start(out=BOFF[:], in_=boff_in[:])
        BDCN = consts.tile([128, 1], F32)
        nc.sync.dma_start(out=BDCN[:], in_=bdcn_in[:])
        WU = consts.tile([128, 1], F32)
        nc.scalar.activation(out=WU[:], in_=BOFF[:], func=AF.Identity,
                             bias=BDCN[:], scale=1.0)
        NEG25 = consts.tile([128, 1], F32)
        nc.vector.memset(NEG25[:], -0.25)

        nh = nr + 2   # z/s/H/OS rows: [r0-1, r0+nr+1)
        nx = nr + 4   # x rows:       [r0-2, r0+nr+2)
        # persistent x slabs (ping-pong): pre-zero once -> pad cols and
        # first-slab halo rows stay zero forever
        XS_pp = []
        for sl in range(2):
            XSz = xpool.tile([128, nx, WP], BF16, tag=f"xs{sl}", name=f"xsz{sl}")
            nc.vector.memset(_f(XSz), 0.0)
            XS_pp.append(XSz)
        # persistent sigmoid-pair tiles: pad cols pre-zeroed once
        SP = []
        for p in range(2):
            SPp = spool.tile([128, nh, WP], BF16, tag=f"sp{p}", name=f"spp{p}")
            nc.vector.memset(SPp[:, :, 0:WP:W + 1], 0.0)
            SP.append(SPp)

        for it in range(cfg.nslab):
            r0 = it * nr

            # ---- load x slab: 4 quarters stacked on partition groups ----
            XS = XS_pp[it % 2]
            for g in range(4):
                i0 = cfg.QH * g + r0   # row in the padded-x frame
                nc.sync.dma_start(
                    out=XS[32 * g:32 * g + 32, :, 1:W + 1],
                    in_=x_in[:, i0:i0 + nx, :])
            XSf = _f(XS[:])

            # ---- conv_off + sigmoid, pairs (q0,q1)->ztA, (q2,q3)->ztB ----
            for zi in range(nh):
                zts = [zpool.tile([128, 512], F32, tag=f"z{p}", name=f"zt{p}")
                       for p in range(2)]
                for t in range(9):
                    kh, kw = t // 3, t % 3
                    base = (zi + kh) * WP + kw
                    for p in range(2):
                        for gq in range(2):
                            g = 2 * p + gq
                            nc.tensor.matmul(
                                zts[p][64 * gq:64 * gq + 64, 1:W + 1],
                                lhsT=WOFF[32 * g:32 * g + 32, t, :],
                                rhs=XSf[32 * g:32 * g + 32, base:base + W],
                                start=(t == 0), stop=(t == 8),
                                tile_position=(32 * g, 64 * gq),
                                skip_group_check=True)
                for p in range(2):
                    nc.scalar.activation(
                        out=SP[p][:, zi, 1:W + 1], in_=zts[p][:, 1:W + 1],
                        func=AF.Sigmoid, bias=BOFF[:], scale=1.0)

            # ---- fold s into quarter-stacked SX/SY (8 sbuf->sbuf DMAs) ----
            SX = spool.tile([128, nh, WP], BF16, tag="sx")
            SY = spool.tile([128, nh, WP], BF16, tag="sy")
            with tc.tile_critical():
                for g in range(4):
                    p, gq = g // 2, g % 2
                    nc.gpsimd.dma_start(
                        out=_f(SX[32 * g:32 * g + 32]),
                        in_=_f(SP[p][64 * gq:64 * gq + 32])).then_inc(fold_sem, 16)
                    nc.gpsimd.dma_start(
                        out=_f(SY[32 * g:32 * g + 32]),
                        in_=_f(SP[p][64 * gq + 32:64 * gq + 64])).then_inc(fold_sem, 16)
                fold_cnt[0] += 128
                nc.gpsimd.wait_ge(fold_sem, fold_cnt[0])

            # ---- displacement maps: dxt = s/2 - 1/4, |dxt| ----
            DX = mpool.tile([128, nh, WP], BF16, tag="dx")
            nc.vector.tensor_scalar(_f(DX), _f(SX), 0.5, -0.25, OP.mult, OP.add)
            ADX = mpool.tile([128, nh, WP], BF16, tag="adx")
            nc.scalar.activation(out=_f(ADX), in_=_f(SX), func=AF.Abs,
                                 bias=NEG25[:], scale=0.5)

            # ---- column diff images on x geometry ----
            Lx = nx * WP
            AR = abpool.tile([128, nx, WP], BF16, tag="ar")
            ARf = _f(AR)
            nc.vector.tensor_tensor(
                ARf[:, 1:Lx - 1], XSf[:, 0:Lx - 2], XSf[:, 2:Lx], OP.subtract)
            BR0 = abpool.tile([128, nx, WP], BF16, tag="br0")
            BR0f = _f(BR0)
            nc.gpsimd.tensor_tensor(
                BR0f[:, 1:Lx - 1], XSf[:, 0:Lx - 2], XSf[:, 2:Lx], OP.add)
            BR = abpool.tile([128, nx, WP], BF16, tag="br")
            BRf = _f(BR)
            nc.vector.scalar_tensor_tensor(
                BRf[:, 1:Lx - 1], in0=XSf[:, 1:Lx - 1], scalar=-2.0,
                in1=BR0f[:, 1:Lx - 1], op0=OP.mult, op1=OP.add)
            for tl in (ARf, BRf):
                nc.vector.memset(tl[:, 0:1], 0.0)
                nc.vector.memset(tl[:, Lx - 1:Lx], 0.0)
            # reflect fixups at image cols 0 / W-1 (padded cols 1 / W)
            nc.vector.memset(AR[:, :, 1], 0.0)
            nc.vector.memset(AR[:, :, W], 0.0)
            nc.vector.tensor_tensor(
                BR[:, :, 1], BR[:, :, 1], XS[:, :, 2], OP.add)
            nc.vector.tensor_tensor(
                BR[:, :, W], BR[:, :, W], XS[:, :, W - 1], OP.add)

            # ---- horizontal interps H(-1), H(0), H(+1) ----
            Lh = nh * WP
            DXf, ADXf = _f(DX), _f(ADX)
            Hs = []
            for dr in (-1, 0, 1):
                off = (1 + dr) * WP
                T1 = hpool.tile([128, nh, WP], BF16, tag="ht1")
                nc.vector.tensor_tensor(
                    _f(T1), DXf, ARf[:, off:off + Lh], OP.mult)
                T2 = hpool.tile([128, nh, WP], BF16, tag="ht2")
                nc.vector.tensor_tensor(
                    _f(T2), ADXf, BRf[:, off:off + Lh], OP.mult)
                Hd = hpool.tile([128, nh, WP], BF16, tag=f"h{dr}")
                nc.vector.tensor_tensor(
                    _f(Hd), _f(T1), XSf[:, off:off + Lh], OP.add)
                nc.vector.tensor_tensor(_f(Hd), _f(Hd), _f(T2), OP.add)
                Hs.append(Hd)
            Hm, H0, Hp = Hs

            # ---- vertical combine ----
            AH = hpool.tile([128, nh, WP], BF16, tag="ah")
            nc.vector.tensor_tensor(_f(AH), _f(Hm), _f(Hp), OP.subtract)
            BH0 = hpool.tile([128, nh, WP], BF16, tag="ht2")
            nc.vector.tensor_tensor(_f(BH0), _f(Hm), _f(Hp), OP.add)
            BH = hpool.tile([128, nh, WP], BF16, tag="bh")
            nc.vector.scalar_tensor_tensor(
                _f(BH), in0=_f(H0), scalar=-2.0, in1=_f(BH0),
                op0=OP.mult, op1=OP.add)
            # y displacement maps (reuse the dx/adx slots)
            DY = mpool.tile([128, nh, WP], BF16, tag="dx")
            nc.vector.tensor_scalar(_f(DY), _f(SY), 0.5, -0.25, OP.mult, OP.add)
            ADY = mpool.tile([128, nh, WP], BF16, tag="adx")
            nc.scalar.activation(out=_f(ADY), in_=_f(SY), func=AF.Abs,
                                 bias=NEG25[:], scale=0.5)
            # reflect fixups at image rows 0 / H-1 (Hm/Hp read zero rows there)
            if it == 0:
                nc.vector.memset(_f(AH[0:32, 1:2, :]), 0.0)
                nc.vector.tensor_tensor(
                    _f(BH[0:32, 1:2, :]), _f(BH[0:32, 1:2, :]),
                    _f(Hp[0:32, 1:2, :]), OP.add)
            if it == cfg.nslab - 1:
                nc.vector.memset(_f(AH[96:128, nr:nr + 1, :]), 0.0)
                nc.vector.tensor_tensor(
                    _f(BH[96:128, nr:nr + 1, :]), _f(BH[96:128, nr:nr + 1, :]),
                    _f(Hm[96:128, nr:nr + 1, :]), OP.add)

            OS = ospool.tile([128, nh, WP], BF16)
            T3 = hpool.tile([128, nh, WP], BF16, tag="ht1")
            nc.vector.tensor_tensor(_f(T3), _f(DY), _f(AH), OP.mult)
            T4 = hpool.tile([128, nh, WP], BF16, tag="ht2")
            nc.vector.tensor_tensor(_f(T4), _f(ADY), _f(BH), OP.mult)
            nc.vector.tensor_tensor(_f(OS), _f(H0), _f(T3), OP.add)
            nc.vector.tensor_tensor(_f(OS), _f(OS), _f(T4), OP.add)
            # sampled outside the image is 0 for the final conv zero-padding
            nc.vector.memset(OS[:, :, 0:WP:W + 1], 0.0)
            if it == 0:
                nc.vector.memset(_f(OS[0:32, 0:1, :]), 0.0)
            if it == cfg.nslab - 1:
                nc.vector.memset(_f(OS[96:128, nr + 1:nr + 2, :]), 0.0)
            OSf = _f(OS)

            # ---- conv_dcn + bias + store (4-row output chunks) ----
            OROWS = 4
            for oc_i in range(nr // OROWS):
                OC = ocpool.tile([128, OROWS, WP], F32)
                for oj in range(OROWS):
                    oi = oc_i * OROWS + oj
                    ot = opool.tile([128, 512], F32)
                    for t in range(9):
                        kh, kw = t // 3, t % 3
                        base = (oi + kh) * WP + kw
                        for g in range(4):
                            nc.tensor.matmul(
                                ot[32 * g:32 * g + 32, 1:W + 1],
                                lhsT=WDCN[32 * g:32 * g + 32, t, :],
                                rhs=OSf[32 * g:32 * g + 32, base:base + W],
                                start=(t == 0), stop=(t == 8),
                                tile_position=(32 * g, 32 * g),
                                skip_group_check=True)
                    nc.scalar.activation(
                        out=OC[:, oj, 1:W + 1], in_=ot[:, 1:W + 1],
                        func=AF.Identity, bias=BDCN[:], scale=1.0)
                with tc.tile_critical():
                    for g in range(4):
                        rr = cfg.QH * g + r0 + oc_i * OROWS
                        nc.gpsimd.dma_start(
                            out=y_out[:, rr:rr + OROWS, :],
                            in_=OC[32 * g:32 * g + 32, :, 1:W + 1]
                        ).then_inc(store_sem, 16)
                    store_cnt[0] += 64
                    nc.gpsimd.wait_ge(store_sem, store_cnt[0])
    if finalize:
        nc.finalize()
    return nc


def prep_weights(w_off, b_off, w_dcn, b_dcn):
    """Host-side packing of conv weights into lhsT tiles, replicated x4."""
    perm = np.concatenate([np.arange(0, 2 * C, 2), np.arange(1, 2 * C, 2)])
    # WOFF[32g+ci, kh*3+kw, m] = w_off[perm[m], ci, kh, kw]
    wo = w_off[perm].astype(np.float32)            # [64, C, 3, 3]
    wo = wo.transpose(1, 2, 3, 0).reshape(C, 9, OC2)   # [ci, tap, m]
    woff = np.tile(wo, (4, 1, 1)).reshape(128, 9 * OC2)
    wd = w_dcn.astype(np.float32).transpose(1, 2, 3, 0).reshape(C, 9, C)
    wdcn = np.tile(wd, (4, 1, 1)).reshape(128, 9 * C)
    boff = np.tile(b_off[perm].astype(np.float32), 2).reshape(128, 1)
    bdcn = np.tile(b_dcn.astype(np.float32), 4).reshape(128, 1)
    return {
        "woff": woff.astype(ml_dtypes.bfloat16),
        "wdcn": wdcn.astype(ml_dtypes.bfloat16),
        "boff": boff.astype(np.float32),
        "bdcn": bdcn.astype(np.float32),
    }


_NC_CACHE = {}


def _get_nc(cfg_key):
    if cfg_key not in _NC_CACHE:
        _NC_CACHE[cfg_key] = build_nc(Cfg(H=cfg_key[0], nr=cfg_key[1]))
    return _NC_CACHE[cfg_key]


def _run(x, w_off, b_off, w_dcn, b_dcn, **spmd_kwargs):
    from concourse.bass_utils import run_bass_kernel_spmd

    B = x.shape[0]
    H = x.shape[2]
    assert x.shape == (B, C, H, H) and B == N_CORES
    nc = _get_nc((H, 8))
    w = prep_weights(np.asarray(w_off), np.asarray(b_off),
                     np.asarray(w_dcn), np.asarray(b_dcn))
    in_maps = []
    for b in range(B):
        m = dict(w)
        xb = np.asarray(x[b]).astype(ml_dtypes.bfloat16)
        m["x"] = np.pad(xb, ((0, 0), (2, 2), (0, 0)))
        in_maps.append(m)
    return run_bass_kernel_spmd(nc, in_maps, list(range(N_CORES)), **spmd_kwargs)


def kernel(x, w_off, b_off, w_dcn, b_dcn):
    res = _run(x, w_off, b_off, w_dcn, b_dcn)
    out = np.stack([res.results[i]["y"] for i in range(N_CORES)], axis=0)
    return out.astype(np.float32)

